# revision 14
# baseline (speedup 1.0000x reference)
"""BoundaryEnhancedLoss on 8 TRN2 NeuronCores — data-parallel over batch.

Math (2-class specialization of the reference):
  d = pred[:,1] - pred[:,0];  pt = sigmoid((2t-1)*d);  ce_pix = -ln(pt)
  focal_pix = 0.25*(1-pt)^2*ce_pix
  boundary bnd = [0 < s < 25], s = 5x5 box-sum of t (zero pad)
  Per-image: S1=sum bnd, S2=sum t*bnd, S3=sum pt*bnd, S4=sum pt*t*bnd
    inter = S4, union = S1 - S3 + 2*S4
  Product sums via the square trick (ACT has free accumulators):
    u = t+bnd:   sum u^2  = sum t + 2*S2 + S1
    v = pt+bnd:  sum v^2  = sum pt^2 + 2*S3 + S1
    m = t*bnd = relu(u-1);  w = pt+m: sum w^2 = sum pt^2 + 2*S4 + S2
  Global: L = sum ln(pt) (ce_sum=-L), F = sum (pt-1)^2*ln(pt) (focal_sum=-F)

Layout: partition p = 32*img + q; chunk r and free block c cover rows
h = 128r + 32c + q. All accum_out columns then separate images by
partition group, so every op runs full-width [128, 2048].
Per-core output stats[128, 4*8]; host reduces partition groups.
"""
import numpy as np
import ml_dtypes
from contextlib import ExitStack

import concourse.bass as bass
import concourse.tile as tile
from concourse import bacc, mybir
from concourse.bass_utils import run_bass_kernel_spmd
from concourse.tile_rust import add_dep_helper

BF16 = mybir.dt.bfloat16
F32 = mybir.dt.float32
Alu = mybir.AluOpType
Act = mybir.ActivationFunctionType

NCORES = 8
BPC = 4          # images per core
H = W = 512
P = 128
Q = 32           # rows per partition-group strip
CB = 4           # h-blocks (free dim) per chunk
NCHUNK = 4       # chunks: h = 128r + 32c + q
NPIX = 32 * H * W
NST = 8          # stat columns per chunk: S1,u2,v2,w2,pt2,L,F,(spare)
STW = NCHUNK * NST


def _band_consts():
    # Block-diagonal 32-bands over q within each 32-partition image group.
    bmain = np.zeros((P, P), dtype=np.float32)
    btop = np.zeros((P, P), dtype=np.float32)   # from block c-1 (q=30,31)
    bbot = np.zeros((P, P), dtype=np.float32)   # from block c+1 (q=0,1)
    for g in range(BPC):
        o = g * Q
        for k in range(Q):
            for m in range(max(0, k - 2), min(Q, k + 3)):
                bmain[o + k, o + m] = 1.0
        # rows h_k = 32(c-1)+q contribute to h_m = 32c+q' iff |q-32-q'|<=2
        btop[o + 30, o + 0] = 1.0
        btop[o + 31, o + 0] = btop[o + 31, o + 1] = 1.0
        # rows h_k = 32(c+1)+q contribute iff |q+32-q'|<=2
        bbot[o + 0, o + 30] = bbot[o + 0, o + 31] = 1.0
        bbot[o + 1, o + 31] = 1.0
    bf = ml_dtypes.bfloat16
    return bmain.astype(bf), btop.astype(bf), bbot.astype(bf)


def build_nc():
    nc = bacc.Bacc("TRN2", target_bir_lowering=False, debug=False,
                   num_devices=NCORES)
    # host pre-arranged: [ch, r, 32*img+q, c, w] / [r, 32*img+q, c, w]
    pred = nc.dram_tensor("pred", [2, NCHUNK, P, CB, W], F32,
                          kind="ExternalInput")
    tgt = nc.dram_tensor("tgt", [NCHUNK, P, CB, W], BF16,
                         kind="ExternalInput")
    bmain = nc.dram_tensor("bmain", [P, P], BF16, kind="ExternalInput")
    btop = nc.dram_tensor("btop", [P, P], BF16, kind="ExternalInput")
    bbot = nc.dram_tensor("bbot", [P, P], BF16, kind="ExternalInput")
    stats = nc.dram_tensor("stats", [P, STW], F32, kind="ExternalOutput")

    with tile.TileContext(nc) as tc, ExitStack() as ctx:
        persist = ctx.enter_context(tc.tile_pool(name="persist", bufs=1))
        work = ctx.enter_context(tc.tile_pool(name="work", bufs=2))
        psum = ctx.enter_context(tc.tile_pool(name="psum", bufs=2, space="PSUM"))

        bias24 = persist.tile([P, 1], F32, tag="bias24")
        nc.gpsimd.memset(bias24[:], -24.0)
        bias_m1 = persist.tile([P, 1], F32, tag="bias_m1")
        nc.gpsimd.memset(bias_m1[:], -1.0)
        bmain_t = persist.tile([P, P], BF16, tag="bmain")
        btop_t = persist.tile([P, P], BF16, tag="btop")
        bbot_t = persist.tile([P, P], BF16, tag="bbot")
        nc.sync.dma_start(bmain_t[:], bmain[:])
        nc.sync.dma_start(btop_t[:], btop[:])
        nc.sync.dma_start(bbot_t[:], bbot[:])

        t_tiles, c_tiles, pt_tiles, st_tiles = [], [], [], []
        for r in range(NCHUNK):
            t_tiles.append(persist.tile([P, CB, W + 4], BF16,
                                        tag=f"t{r}", name=f"t{r}"))
            c_tiles.append(persist.tile([P, CB, W], BF16,
                                        tag=f"c{r}", name=f"c{r}"))
            pt_tiles.append(persist.tile([P, CB, W], BF16,
                                         tag=f"pt{r}", name=f"pt{r}"))
            st_tiles.append(persist.tile([P, NST], F32,
                                         tag=f"st{r}", name=f"st{r}"))
            nc.gpsimd.memset(st_tiles[r][:], 0.0)

        # ---- Phase 1 (interleaved per r): t load + W-conv + pred load +
        # sigmoid chain. Sigmoids run early so the single table switch to
        # the natural_log set (which also contains relu/square) happens once.
        sig_insts = []
        for r in range(NCHUNK):
            tr, cr, ptr = t_tiles[r], c_tiles[r], pt_tiles[r]
            nc.gpsimd.memset(tr[:, :, 0:2], 0.0)
            nc.gpsimd.memset(tr[:, :, W + 2:W + 4], 0.0)
            nc.sync.dma_start(tr[:, :, 2:W + 2], tgt[r])
            a = work.tile([P, CB, W + 3], BF16, tag="wca")
            nc.gpsimd.tensor_tensor(a[:], tr[:, :, 0:W + 3], tr[:, :, 1:W + 4],
                                    op=Alu.add)
            b2 = work.tile([P, CB, W], BF16, tag="wcb")
            nc.gpsimd.tensor_tensor(b2[:], a[:, :, 0:W], a[:, :, 2:W + 2],
                                    op=Alu.add)
            nc.vector.tensor_tensor(cr[:], b2[:], tr[:, :, 4:W + 4], op=Alu.add)

            p0 = work.tile([P, CB, W], F32, tag="p0")
            p1 = work.tile([P, CB, W], F32, tag="p1")
            nc.sync.dma_start(p0[:], pred[0, r])
            nc.sync.dma_start(p1[:], pred[1, r])
            d = work.tile([P, CB, W], BF16, tag="d")
            nc.vector.tensor_tensor(d[:], p1[:], p0[:], op=Alu.subtract)
            ht2 = work.tile([P, CB, W], BF16, tag="ht2")
            nc.vector.tensor_scalar(ht2[:], tr[:, :, 2:W + 2], 0.5, 2.0,
                                    op0=Alu.subtract, op1=Alu.mult)
            hs = work.tile([P, CB, W], BF16, tag="hs")
            nc.vector.tensor_tensor(hs[:], ht2[:], d[:], op=Alu.mult)
            sig_insts.append(nc.scalar.activation(ptr[:], hs[:], Act.Sigmoid))

        # ---- Phase 2 (per r): band matmuls, boundary, square-trick sums ----
        for r in range(NCHUNK):
            tr, cr, ptr, st = t_tiles[r], c_tiles[r], pt_tiles[r], st_tiles[r]
            s = psum.tile([P, CB, W], F32, tag="s")
            for c in range(CB):
                nc.tensor.matmul(s[:, c, :], bmain_t[:], cr[:, c, :],
                                 start=True, stop=False)
                if c > 0:
                    rhs_top = cr[:, c - 1, :]
                elif r > 0:
                    rhs_top = c_tiles[r - 1][:, CB - 1, :]
                else:
                    rhs_top = None
                if rhs_top is not None:
                    nc.tensor.matmul(s[:, c, :], btop_t[:], rhs_top,
                                     start=False,
                                     stop=(c == CB - 1 and r == NCHUNK - 1))
                if c < CB - 1:
                    rhs_bot = cr[:, c + 1, :]
                elif r < NCHUNK - 1:
                    rhs_bot = c_tiles[r + 1][:, 0, :]
                else:
                    rhs_bot = None
                if rhs_bot is not None:
                    nc.tensor.matmul(s[:, c, :], bbot_t[:], rhs_bot,
                                     start=False, stop=True)

            ero = work.tile([P, CB, W], BF16, tag="ero")
            nc.scalar.activation(ero[:], s[:], Act.Relu, bias=bias24[:])
            bnd = work.tile([P, CB, W], BF16, tag="bnd")
            nc.vector.scalar_tensor_tensor(
                bnd[:], s[:], 0.5, ero[:],
                op0=Alu.is_ge, op1=Alu.subtract, accum_out=st[:, 0:1])
            t_ap = tr[:, :, 2:W + 2]
            u = work.tile([P, CB, W], BF16, tag="u")
            nc.gpsimd.tensor_tensor(u[:], t_ap, bnd[:], op=Alu.add)
            nc.scalar.activation(work.tile([P, CB, W], BF16, tag="jq", name="ju", bufs=1),
                                 u[:], Act.Square, accum_out=st[:, 1:2])
            m = work.tile([P, CB, W], BF16, tag="m")
            nc.vector.tensor_scalar(m[:], u[:], 1.0, 0.0,
                                    op0=Alu.subtract, op1=Alu.max)
            v = work.tile([P, CB, W], BF16, tag="v")
            nc.vector.tensor_tensor(v[:], ptr[:], bnd[:], op=Alu.add)
            nc.scalar.activation(work.tile([P, CB, W], BF16, tag="jq", name="jv", bufs=1),
                                 v[:], Act.Square, accum_out=st[:, 2:3])
            w = work.tile([P, CB, W], BF16, tag="w")
            nc.vector.tensor_tensor(w[:], ptr[:], m[:], op=Alu.add)
            nc.scalar.activation(work.tile([P, CB, W], BF16, tag="jq", name="jw", bufs=1),
                                 w[:], Act.Square, accum_out=st[:, 3:4])
            nc.scalar.activation(work.tile([P, CB, W], BF16, tag="jq", name="jp", bufs=1),
                                 ptr[:], Act.Square, accum_out=st[:, 4:5])

        # ---- Loop 3: ln(pt), focal ----
        for r in range(NCHUNK):
            ptr, st = pt_tiles[r], st_tiles[r]
            lnp = work.tile([P, CB, W], BF16, tag="lnp")
            li = nc.scalar.activation(lnp[:], ptr[:], Act.Ln,
                                      accum_out=st[:, 5:6])
            add_dep_helper(li.ins, sig_insts[-1].ins, sync=False,
                           reason="group ln-set ops after sigmoid-set ops")
            sq = work.tile([P, CB, W], BF16, tag="sq")
            nc.scalar.activation(sq[:], ptr[:], Act.Square, bias=bias_m1[:])
            fo = work.tile([P, CB, W], BF16, tag="fo")
            nc.vector.scalar_tensor_tensor(
                fo[:], sq[:], 1.0, lnp[:], op0=Alu.mult, op1=Alu.mult,
                accum_out=st[:, 6:7])

        for r in range(NCHUNK):
            nc.sync.dma_start(stats[:, bass.ts(r, NST)], st_tiles[r][:])

    nc.compile()
    return nc


_NC = None


def _get_nc():
    global _NC
    if _NC is None:
        _NC = build_nc()
    return _NC


def _host_combine(stats_all, sum_t):
    """stats_all: 8x [128, 32] f32; sum_t: [32] per-image sum of target."""
    S1 = np.zeros(32, np.float64)
    U2 = np.zeros(32, np.float64)
    V2 = np.zeros(32, np.float64)
    W2 = np.zeros(32, np.float64)
    P2 = np.zeros(32, np.float64)
    L = 0.0
    F = 0.0
    for core, stm in enumerate(stats_all):
        # [128, 4r, 8] -> per-image groups of 32 partitions
        g = stm.astype(np.float64).reshape(BPC, Q, NCHUNK, NST).sum(axis=(1, 2))
        for i in range(BPC):
            gi = core * BPC + i
            S1[gi] += g[i, 0]
            U2[gi] += g[i, 1]
            V2[gi] += g[i, 2]
            W2[gi] += g[i, 3]
            P2[gi] += g[i, 4]
        L += g[:, 5].sum()
        F += g[:, 6].sum()
    S2 = (U2 - sum_t - S1) / 2.0
    S3 = (V2 - P2 - S1) / 2.0
    S4 = (W2 - P2 - S2) / 2.0
    ce_loss = (-L) / NPIX
    focal = 0.25 * (-F) / NPIX
    inter = S4
    union = S1 - S3 + 2.0 * S4
    dice = 2.0 * inter / (union + 1e-8)
    bdice = 1.0 - dice.mean()
    return np.float32(ce_loss + focal + bdice)


def run_cores(pred, target, trace=False):
    nc = _get_nc()
    bmain, btop, bbot = _band_consts()
    tgt_f = target.astype(np.float32)
    sum_t = tgt_f.astype(np.float64).sum(axis=(1, 2))
    pred = np.asarray(pred, dtype=np.float32)
    in_maps = []
    for core in range(NCORES):
        sl = slice(core * BPC, (core + 1) * BPC)
        # [b, ch, 128r+32c+q, w] -> [ch, r, 32b+q, c, w]
        pl = (pred[sl].reshape(BPC, 2, NCHUNK, CB, Q, W)
              .transpose(1, 2, 0, 4, 3, 5).reshape(2, NCHUNK, P, CB, W))
        tl = (tgt_f[sl].reshape(BPC, NCHUNK, CB, Q, W)
              .transpose(1, 0, 3, 2, 4).reshape(NCHUNK, P, CB, W)
              .astype(ml_dtypes.bfloat16))
        in_maps.append({
            "pred": np.ascontiguousarray(pl),
            "tgt": np.ascontiguousarray(tl),
            "bmain": bmain,
            "btop": btop,
            "bbot": bbot,
        })
    res = run_bass_kernel_spmd(nc, in_maps, list(range(NCORES)), trace=trace)
    stats_all = [res.results[c]["stats"] for c in range(NCORES)]
    return stats_all, sum_t, res.exec_time_ns


def kernel(pred, target):
    stats_all, sum_t, _ = run_cores(pred, target, trace=False)
    return _host_combine(stats_all, sum_t)


# revision 15
# speedup vs baseline: 1.0890x; 1.0890x over previous
"""BoundaryEnhancedLoss on 8 TRN2 NeuronCores — data-parallel over batch.

Math (2-class specialization of the reference):
  d = pred[:,1] - pred[:,0];  pt = sigmoid((2t-1)*d);  ce_pix = -ln(pt)
  focal_pix = 0.25*(1-pt)^2*ce_pix
  boundary bnd = [0 < s < 25], s = 5x5 box-sum of t (zero pad)
  Per-image: S1=sum bnd, S2=sum t*bnd, S3=sum pt*bnd, S4=sum pt*t*bnd
    inter = S4, union = S1 - S3 + 2*S4
  Product sums via the square trick (ACT has free accumulators):
    u = t+bnd:   sum u^2  = sum t + 2*S2 + S1
    v = pt+bnd:  sum v^2  = sum pt^2 + 2*S3 + S1
    m = t*bnd = relu(u-1);  w = pt+m: sum w^2 = sum pt^2 + 2*S4 + S2
  Global: L = sum ln(pt) (ce_sum=-L), F = sum (pt-1)^2*ln(pt) (focal_sum=-F)

Layout: partition p = 32*img + q; chunk r and free block c cover rows
h = 128r + 32c + q. All accum_out columns then separate images by
partition group, so every op runs full-width [128, 2048].
Per-core output stats[128, 4*8]; host reduces partition groups.
"""
import numpy as np
import ml_dtypes
from contextlib import ExitStack

import concourse.bass as bass
import concourse.tile as tile
from concourse import bacc, mybir
from concourse.bass_utils import run_bass_kernel_spmd
from concourse.tile_rust import add_dep_helper

BF16 = mybir.dt.bfloat16
F32 = mybir.dt.float32
Alu = mybir.AluOpType
Act = mybir.ActivationFunctionType

NCORES = 8
BPC = 4          # images per core
H = W = 512
P = 128
Q = 32           # rows per partition-group strip
CB = 4           # h-blocks (free dim) per chunk
NCHUNK = 4       # chunks: h = 128r + 32c + q
NPIX = 32 * H * W
NST = 8          # stat columns per chunk: S1,u2,v2,w2,pt2,L,F,(spare)
STW = NCHUNK * NST


def _band_consts():
    # Block-diagonal 32-bands over q within each 32-partition image group.
    bmain = np.zeros((P, P), dtype=np.float32)
    btop = np.zeros((P, P), dtype=np.float32)   # from block c-1 (q=30,31)
    bbot = np.zeros((P, P), dtype=np.float32)   # from block c+1 (q=0,1)
    for g in range(BPC):
        o = g * Q
        for k in range(Q):
            for m in range(max(0, k - 2), min(Q, k + 3)):
                bmain[o + k, o + m] = 1.0
        # rows h_k = 32(c-1)+q contribute to h_m = 32c+q' iff |q-32-q'|<=2
        btop[o + 30, o + 0] = 1.0
        btop[o + 31, o + 0] = btop[o + 31, o + 1] = 1.0
        # rows h_k = 32(c+1)+q contribute iff |q+32-q'|<=2
        bbot[o + 0, o + 30] = bbot[o + 0, o + 31] = 1.0
        bbot[o + 1, o + 31] = 1.0
    bf = ml_dtypes.bfloat16
    return bmain.astype(bf), btop.astype(bf), bbot.astype(bf)


def build_nc():
    nc = bacc.Bacc("TRN2", target_bir_lowering=False, debug=False,
                   num_devices=NCORES)
    # host pre-arranged: [ch, r, 32*img+q, c, w] / [r, 32*img+q, c, w]
    pred = nc.dram_tensor("pred", [2, NCHUNK, P, CB, W], F32,
                          kind="ExternalInput")
    tgt = nc.dram_tensor("tgt", [NCHUNK, P, CB, W], BF16,
                         kind="ExternalInput")
    bmain = nc.dram_tensor("bmain", [P, P], BF16, kind="ExternalInput")
    btop = nc.dram_tensor("btop", [P, P], BF16, kind="ExternalInput")
    bbot = nc.dram_tensor("bbot", [P, P], BF16, kind="ExternalInput")
    stats = nc.dram_tensor("stats", [P, STW], F32, kind="ExternalOutput")

    with tile.TileContext(nc) as tc, ExitStack() as ctx:
        persist = ctx.enter_context(tc.tile_pool(name="persist", bufs=1))
        work = ctx.enter_context(tc.tile_pool(name="work", bufs=2))
        psum = ctx.enter_context(tc.tile_pool(name="psum", bufs=2, space="PSUM"))

        bias24 = persist.tile([P, 1], F32, tag="bias24")
        nc.gpsimd.memset(bias24[:], -24.0)
        bias_m1 = persist.tile([P, 1], F32, tag="bias_m1")
        nc.gpsimd.memset(bias_m1[:], -1.0)
        bmain_t = persist.tile([P, P], BF16, tag="bmain")
        btop_t = persist.tile([P, P], BF16, tag="btop")
        bbot_t = persist.tile([P, P], BF16, tag="bbot")
        nc.sync.dma_start(bmain_t[:], bmain[:])
        nc.sync.dma_start(btop_t[:], btop[:])
        nc.sync.dma_start(bbot_t[:], bbot[:])

        t_tiles, c_tiles, pt_tiles, st_tiles = [], [], [], []
        for r in range(NCHUNK):
            t_tiles.append(persist.tile([P, CB, W + 4], BF16,
                                        tag=f"t{r}", name=f"t{r}"))
            c_tiles.append(persist.tile([P, CB, W], BF16,
                                        tag=f"c{r}", name=f"c{r}"))
            pt_tiles.append(persist.tile([P, CB, W], BF16,
                                         tag=f"pt{r}", name=f"pt{r}"))
            st_tiles.append(persist.tile([P, NST], F32,
                                         tag=f"st{r}", name=f"st{r}"))
            nc.gpsimd.memset(st_tiles[r][:], 0.0)

        # ---- Phase 1 (interleaved per r): t load + W-conv + pred load +
        # sigmoid chain. Sigmoids run early so the single table switch to
        # the natural_log set (which also contains relu/square) happens once.
        sig_insts = []
        for r in range(NCHUNK):
            tr, cr, ptr = t_tiles[r], c_tiles[r], pt_tiles[r]
            nc.gpsimd.memset(tr[:, :, 0:2], 0.0)
            nc.gpsimd.memset(tr[:, :, W + 2:W + 4], 0.0)
            nc.sync.dma_start(tr[:, :, 2:W + 2], tgt[r])
            a = work.tile([P, CB, W + 3], BF16, tag="wca")
            nc.gpsimd.tensor_tensor(a[:], tr[:, :, 0:W + 3], tr[:, :, 1:W + 4],
                                    op=Alu.add)
            b2 = work.tile([P, CB, W], BF16, tag="wcb")
            nc.gpsimd.tensor_tensor(b2[:], a[:, :, 0:W], a[:, :, 2:W + 2],
                                    op=Alu.add)
            nc.vector.tensor_tensor(cr[:], b2[:], tr[:, :, 4:W + 4], op=Alu.add)

            p0 = work.tile([P, CB, W], F32, tag="p0")
            p1 = work.tile([P, CB, W], F32, tag="p1")
            nc.sync.dma_start(p0[:], pred[0, r])
            nc.sync.dma_start(p1[:], pred[1, r])
            d = work.tile([P, CB, W], BF16, tag="d")
            nc.vector.tensor_tensor(d[:], p1[:], p0[:], op=Alu.subtract)
            ht2 = work.tile([P, CB, W], BF16, tag="ht2")
            nc.vector.tensor_scalar(ht2[:], tr[:, :, 2:W + 2], 0.5, 2.0,
                                    op0=Alu.subtract, op1=Alu.mult)
            hs = work.tile([P, CB, W], BF16, tag="hs")
            nc.vector.tensor_tensor(hs[:], ht2[:], d[:], op=Alu.mult)
            sig_insts.append(nc.scalar.activation(ptr[:], hs[:], Act.Sigmoid))

        # ---- Phase 2 (per r): band matmuls, boundary, square-trick sums ----
        for r in range(NCHUNK):
            tr, cr, ptr, st = t_tiles[r], c_tiles[r], pt_tiles[r], st_tiles[r]
            s = psum.tile([P, CB, W], F32, tag="s")
            for c in range(CB):
                nc.tensor.matmul(s[:, c, :], bmain_t[:], cr[:, c, :],
                                 start=True, stop=False)
                if c > 0:
                    rhs_top = cr[:, c - 1, :]
                elif r > 0:
                    rhs_top = c_tiles[r - 1][:, CB - 1, :]
                else:
                    rhs_top = None
                if rhs_top is not None:
                    nc.tensor.matmul(s[:, c, :], btop_t[:], rhs_top,
                                     start=False,
                                     stop=(c == CB - 1 and r == NCHUNK - 1))
                if c < CB - 1:
                    rhs_bot = cr[:, c + 1, :]
                elif r < NCHUNK - 1:
                    rhs_bot = c_tiles[r + 1][:, 0, :]
                else:
                    rhs_bot = None
                if rhs_bot is not None:
                    nc.tensor.matmul(s[:, c, :], bbot_t[:], rhs_bot,
                                     start=False, stop=True)

            ero = work.tile([P, CB, W], BF16, tag="ero")
            nc.scalar.activation(ero[:], s[:], Act.Relu, bias=bias24[:])
            bnd = work.tile([P, CB, W], BF16, tag="bnd")
            nc.vector.scalar_tensor_tensor(
                bnd[:], s[:], 0.5, ero[:],
                op0=Alu.is_ge, op1=Alu.subtract, accum_out=st[:, 0:1])
            t_ap = tr[:, :, 2:W + 2]
            u = work.tile([P, CB, W], BF16, tag="u")
            nc.vector.tensor_tensor(u[:], t_ap, bnd[:], op=Alu.add)
            nc.scalar.activation(work.tile([P, CB, W], BF16, tag="jq", name="ju", bufs=1),
                                 u[:], Act.Square, accum_out=st[:, 1:2])
            m = work.tile([P, CB, W], BF16, tag="m")
            nc.vector.tensor_scalar(m[:], u[:], 1.0, 0.0,
                                    op0=Alu.subtract, op1=Alu.max)
            v = work.tile([P, CB, W], BF16, tag="v")
            nc.vector.tensor_tensor(v[:], ptr[:], bnd[:], op=Alu.add)
            nc.scalar.activation(work.tile([P, CB, W], BF16, tag="jq", name="jv", bufs=1),
                                 v[:], Act.Square, accum_out=st[:, 2:3])
            w = work.tile([P, CB, W], BF16, tag="w")
            nc.vector.tensor_tensor(w[:], ptr[:], m[:], op=Alu.add)
            nc.scalar.activation(work.tile([P, CB, W], BF16, tag="jq", name="jw", bufs=1),
                                 w[:], Act.Square, accum_out=st[:, 3:4])
            nc.scalar.activation(work.tile([P, CB, W], BF16, tag="jq", name="jp", bufs=1),
                                 ptr[:], Act.Square, accum_out=st[:, 4:5])

        # ---- Loop 3: ln(pt), focal ----
        for r in range(NCHUNK):
            ptr, st = pt_tiles[r], st_tiles[r]
            lnp = work.tile([P, CB, W], BF16, tag="lnp")
            li = nc.scalar.activation(lnp[:], ptr[:], Act.Ln,
                                      accum_out=st[:, 5:6])
            add_dep_helper(li.ins, sig_insts[-1].ins, sync=False,
                           reason="group ln-set ops after sigmoid-set ops")
            sq = work.tile([P, CB, W], BF16, tag="sq")
            nc.scalar.activation(sq[:], ptr[:], Act.Square, bias=bias_m1[:])
            fo = work.tile([P, CB, W], BF16, tag="fo")
            nc.vector.scalar_tensor_tensor(
                fo[:], sq[:], 1.0, lnp[:], op0=Alu.mult, op1=Alu.mult,
                accum_out=st[:, 6:7])

        for r in range(NCHUNK):
            nc.sync.dma_start(stats[:, bass.ts(r, NST)], st_tiles[r][:])

    nc.compile()
    return nc


_NC = None


def _get_nc():
    global _NC
    if _NC is None:
        _NC = build_nc()
    return _NC


def _host_combine(stats_all, sum_t):
    """stats_all: 8x [128, 32] f32; sum_t: [32] per-image sum of target."""
    S1 = np.zeros(32, np.float64)
    U2 = np.zeros(32, np.float64)
    V2 = np.zeros(32, np.float64)
    W2 = np.zeros(32, np.float64)
    P2 = np.zeros(32, np.float64)
    L = 0.0
    F = 0.0
    for core, stm in enumerate(stats_all):
        # [128, 4r, 8] -> per-image groups of 32 partitions
        g = stm.astype(np.float64).reshape(BPC, Q, NCHUNK, NST).sum(axis=(1, 2))
        for i in range(BPC):
            gi = core * BPC + i
            S1[gi] += g[i, 0]
            U2[gi] += g[i, 1]
            V2[gi] += g[i, 2]
            W2[gi] += g[i, 3]
            P2[gi] += g[i, 4]
        L += g[:, 5].sum()
        F += g[:, 6].sum()
    S2 = (U2 - sum_t - S1) / 2.0
    S3 = (V2 - P2 - S1) / 2.0
    S4 = (W2 - P2 - S2) / 2.0
    ce_loss = (-L) / NPIX
    focal = 0.25 * (-F) / NPIX
    inter = S4
    union = S1 - S3 + 2.0 * S4
    dice = 2.0 * inter / (union + 1e-8)
    bdice = 1.0 - dice.mean()
    return np.float32(ce_loss + focal + bdice)


def run_cores(pred, target, trace=False):
    nc = _get_nc()
    bmain, btop, bbot = _band_consts()
    tgt_f = target.astype(np.float32)
    sum_t = tgt_f.astype(np.float64).sum(axis=(1, 2))
    pred = np.asarray(pred, dtype=np.float32)
    in_maps = []
    for core in range(NCORES):
        sl = slice(core * BPC, (core + 1) * BPC)
        # [b, ch, 128r+32c+q, w] -> [ch, r, 32b+q, c, w]
        pl = (pred[sl].reshape(BPC, 2, NCHUNK, CB, Q, W)
              .transpose(1, 2, 0, 4, 3, 5).reshape(2, NCHUNK, P, CB, W))
        tl = (tgt_f[sl].reshape(BPC, NCHUNK, CB, Q, W)
              .transpose(1, 0, 3, 2, 4).reshape(NCHUNK, P, CB, W)
              .astype(ml_dtypes.bfloat16))
        in_maps.append({
            "pred": np.ascontiguousarray(pl),
            "tgt": np.ascontiguousarray(tl),
            "bmain": bmain,
            "btop": btop,
            "bbot": bbot,
        })
    res = run_bass_kernel_spmd(nc, in_maps, list(range(NCORES)), trace=trace)
    stats_all = [res.results[c]["stats"] for c in range(NCORES)]
    return stats_all, sum_t, res.exec_time_ns


def kernel(pred, target):
    stats_all, sum_t, _ = run_cores(pred, target, trace=False)
    return _host_combine(stats_all, sum_t)
